# revision 9
# baseline (speedup 1.0000x reference)
"""Trainium2 Bass kernel for nn_CrossAttention (self-attention, B=2, S=2048,
16 heads x 64 dim, d_model=1024).

Sharding: batch*heads across 8 cores -> each core owns 2 heads for both
batches. Each core receives the full (pre-transposed, fp16) hidden states and
its 128-column slice of w_q/w_k/w_v (w_q pre-scaled by 1/sqrt(64)) plus its
128-row slice of w_o. Cores emit fp16 partial outputs [4096, 1024] (the w_o
contraction over the core's 128 inner dims); the host sums the partials in
fp32 and adds b_o.

Per-core dataflow (all matmuls fp16 operands, fp32 PSUM accumulation):
  1. Q^T/K^T/V^T [128, 4096] = w.T @ hs^T   (feature-major projections)
  2. V^T -> V plain [seq, 64] per k-tile via PE transpose, ones column
     appended (softmax denominator rides the ctx matmul for free)
  3. per (batch, q-chunk): scores^T[k, q] for BOTH heads packed into one
     PSUM tile via tile_position row groups (the two K=64 matmuls run
     concurrently on the PE array), one exp per packed tile on ACT,
     ctx^T[d, q] += [v | 1].T @ p^T per head
  4. normalize: DVE reciprocal -> gpsimd partition_broadcast -> DVE multiply
  5. out[q, 1024] = ctx^T.T @ w_o, interleaved into the next attention block

PSUM bank budget (8 banks of 2KB):
  tag A (scores, [128,1024] f32 = 2 banks) x2 bufs          -> 4 banks
  tag C (ctx accumulators, [65,512] f32 = 1 bank) x2 bufs   -> 2 banks
  tag B (proj/transpose/out transients, 1 bank) x2 bufs     -> 2 banks
With ctx in its own tag, projection bursts never pin the ctx slots, so
attention (ACT-bound) and projections (PE-bound) overlap.
"""
import numpy as np

HEADS = 16
DIM_HEAD = 64
QUERY_DIM = 1024
SCALE = DIM_HEAD ** -0.5
B, S = 2, 2048
NSEQ = B * S              # 4096
N_CORES = 8
FEAT = 128                # 2 heads x 64 per core
KT = S // 128             # 16 k-tiles per batch

_nc_cache = {}


def _build(n_iters=1, unroll=1):
    import concourse.bass as bass
    import concourse.tile as tile
    from concourse import bacc, mybir
    from concourse.masks import make_identity

    F32 = mybir.dt.float32
    F16 = mybir.dt.float16
    AF = mybir.ActivationFunctionType

    nc = bacc.Bacc("TRN2", target_bir_lowering=False)

    hst = nc.dram_tensor("hst", [QUERY_DIM, NSEQ], F16, kind="ExternalInput")
    wq = nc.dram_tensor("wq", [QUERY_DIM, FEAT], F16, kind="ExternalInput")
    wk = nc.dram_tensor("wk", [QUERY_DIM, FEAT], F16, kind="ExternalInput")
    wv = nc.dram_tensor("wv", [QUERY_DIM, FEAT], F16, kind="ExternalInput")
    wo = nc.dram_tensor("wo", [FEAT, QUERY_DIM], F16, kind="ExternalInput")
    out = nc.dram_tensor("out", [NSEQ, QUERY_DIM], F16, kind="ExternalOutput")

    with tile.TileContext(nc) as tc:
        with (
            tc.tile_pool(name="sb", bufs=1) as sb,
            tc.tile_pool(name="ps", bufs=1, space="PSUM") as ps,
        ):
            wq_sb = sb.tile([128, 8, FEAT], F16)
            wk_sb = sb.tile([128, 8, FEAT], F16)
            wv_sb = sb.tile([128, 8, FEAT], F16)
            wo_sb = sb.tile([128, QUERY_DIM], F16)
            ident = sb.tile([128, 128], F16)

            qT_sb = sb.tile([128, NSEQ], F16)
            kT_sb = sb.tile([128, NSEQ], F16)
            ctxn_sb = sb.tile([128, NSEQ], F16)
            v65 = sb.tile([128, 4 * KT, 65], F16)
            nc.gpsimd.memset(v65[:, :, 64:65], 1.0)

            hst_src = hst.ap().rearrange("(kt p) n -> p kt n", p=128)
            w_sbs = {"q": wq_sb, "k": wk_sb, "v": wv_sb}

            def proj_half(proj, g, half, hst_t):
                """One 512-col half of one projection for seq-group g.
                For v, also transposes the 4 fresh 128-col tiles into v65."""
                g0 = g * 1024
                h0 = half * 512
                vT_t = None
                if proj == "v":
                    vT_t = sb.tile([128, 512], F16, tag="vT_t", bufs=2,
                                   name=f"vT_t{g}_{half}")
                p_p = ps.tile([128, 512], F32, tag="B", bufs=2,
                              name=f"p_{proj}{g}_{half}")
                for kt in range(8):
                    nc.tensor.matmul(
                        p_p[:], w_sbs[proj][:, kt, :],
                        hst_t[:, kt, h0:h0 + 512],
                        start=(kt == 0), stop=(kt == 7),
                    )
                if proj == "q":
                    nc.vector.tensor_copy(qT_sb[:, g0 + h0:g0 + h0 + 512], p_p[:])
                elif proj == "k":
                    nc.vector.tensor_copy(kT_sb[:, g0 + h0:g0 + h0 + 512], p_p[:])
                else:
                    nc.vector.tensor_copy(vT_t[:], p_p[:])
                    for c in range(4):
                        ci = g * 8 + half * 4 + c
                        b_i, kt_loc = ci // 16, ci % 16
                        p_tr = ps.tile([128, 128], F16, tag="B", bufs=2,
                                       name=f"p_tr{ci}")
                        nc.tensor.transpose(
                            p_tr[:], vT_t[:, c * 128:(c + 1) * 128], ident[:])
                        # one strided copy fills both heads' v65 slots
                        v65_4d = v65.rearrange("p (pr kt) c -> p pr kt c", pr=4)
                        dst = v65_4d[:, b_i * 2:(b_i + 1) * 2, kt_loc, 0:64]
                        nc.vector.tensor_copy(
                            dst, p_tr.rearrange("p (h d) -> p h d", h=2))

            def load_group(g, split_first=False):
                hst_t = sb.tile([128, 8, 1024], F16, tag="hst_t", bufs=2,
                                name=f"hst_t{g}")
                g0 = g * 1024
                if split_first:
                    # per-kt DMAs for the first 512 cols: the startup
                    # projection chain consumes kt i as soon as DMA i lands
                    for kt in range(8):
                        nc.sync.dma_start(hst_t[:, kt, 0:512],
                                          hst_src[:, kt, g0:g0 + 512])
                    nc.sync.dma_start(hst_t[:, :, 512:1024],
                                      hst_src[:, :, g0 + 512:g0 + 1024])
                else:
                    nc.sync.dma_start(hst_t[:], hst_src[:, :, g0:g0 + 1024])
                return hst_t

            def proj_group(g, hst_t):
                for half in range(2):
                    for proj in ("k", "q", "v"):
                        proj_half(proj, g, half, hst_t)

            def out_qt(qt):
                """One 128-row tile of the final projection."""
                t0 = qt * 128
                o_sb = sb.tile([128, 1024], F16, tag="o_sb", bufs=6,
                               name=f"o_sb{qt}")
                for c in range(2):
                    p_o = ps.tile([128, 512], F32, tag="B", bufs=2,
                                  name=f"p_o{qt}_{c}")
                    nc.tensor.matmul(
                        p_o[:], ctxn_sb[:, t0:t0 + 128],
                        wo_sb[:, c * 512:(c + 1) * 512],
                        start=True, stop=True)
                    nc.vector.tensor_copy(o_sb[:, c * 512:(c + 1) * 512], p_o[:])
                nc.gpsimd.dma_start(out[t0:t0 + 128, :], o_sb[:])

            def attn_part(b_i, cc, kts, p_ctx, out_qts=()):
                """Score+exp+ctx for kt in kts, (batch, 512-q-chunk cc),
                head-packed scores; out_qts interleaved."""
                s0 = b_i * S
                q0 = s0 + cc * 512
                oq = list(out_qts)
                kts = list(kts)
                # spread out-tiles across the kt steps, starting at kt>=1
                pop_at = set()
                if oq:
                    navail = max(len(kts) - 1, 1)
                    step = max(navail // len(oq), 1)
                    for i in range(len(oq)):
                        pop_at.add(kts[min(1 + i * step, len(kts) - 1)])
                for kt in kts:
                    k0 = s0 + kt * 128
                    p_s = ps.tile([128, 1024], F32, tag="A", bufs=2,
                                  name=f"p_s{b_i}_{cc}_{kt}")
                    # head-packed: head h uses PE row group h*64, writes its
                    # own PSUM bank within the shared tile
                    for h in range(2):
                        hp = slice(h * 64, (h + 1) * 64)
                        nc.tensor.matmul(
                            p_s[:, h * 512:(h + 1) * 512],
                            kT_sb[hp, k0:k0 + 128],
                            qT_sb[hp, q0:q0 + 512],
                            start=True, stop=True,
                            tile_position=(h * 64, 0),
                        )
                    pT = sb.tile([128, 1024], F16, tag="pT", bufs=12,
                                 name=f"pT{b_i}_{cc}_{kt}")
                    nc.scalar.activation(pT[:], p_s[:], AF.Exp)
                    for h in range(2):
                        nc.tensor.matmul(
                            p_ctx[h][:],
                            v65[:, (b_i * 2 + h) * 16 + kt, :],
                            pT[:, h * 512:(h + 1) * 512],
                            start=(kt == 0), stop=(kt == KT - 1),
                        )
                    if kt in pop_at and oq:
                        out_qt(oq.pop(0))
                while oq:
                    out_qt(oq.pop(0))

            def ctx_tiles(b_i, cc):
                return [ps.tile([65, 512], F32, tag="C", bufs=2,
                                name=f"p_ctx{b_i}_{cc}_{h}")
                        for h in range(2)]

            def norm(b_i, cc, p_ctx):
                s0 = b_i * S
                q0 = s0 + cc * 512
                for h in range(2):
                    hp = slice(h * 64, (h + 1) * 64)
                    pc = p_ctx[h]
                    recip = sb.tile([1, 512], F32, tag="recip", bufs=4,
                                    name=f"recip{b_i}_{cc}_{h}")
                    nc.vector.reciprocal(recip[:], pc[64:65, :])
                    rbc_sb = sb.tile([64, 512], F32, tag="rbc", bufs=4,
                                     name=f"rbc{b_i}_{cc}_{h}")
                    nc.gpsimd.partition_broadcast(rbc_sb[:], recip[:])
                    nc.vector.tensor_mul(
                        ctxn_sb[hp, q0:q0 + 512], pc[0:64, :], rbc_sb[:])

            def attn_block(b_i, cc, out_qts, ctx=None, kts=None):
                p_ctx = ctx if ctx is not None else ctx_tiles(b_i, cc)
                attn_part(b_i, cc, kts if kts is not None else range(KT),
                          p_ctx, out_qts)
                if kts is None or kts[-1] == KT - 1:
                    norm(b_i, cc, p_ctx)
                return p_ctx

            # ---- emission order. Program order defines dataflow (producers
            # strictly before consumers); Tile pulls later-emitted independent
            # work into engine idle slots. Fine-grained start: attention on
            # (0,0) begins after the first half-projections of group 0. Out
            # tiles lag their chunk by one block; ctx slots (tag C) free at
            # each norm so the next chunk's ctx never waits on proj psum. ----
            def emit_body():
                nc.sync.dma_start(wk_sb[:], wk.ap().rearrange("(kt p) m -> p kt m", p=128))
                nc.sync.dma_start(wq_sb[:], wq.ap().rearrange("(kt p) m -> p kt m", p=128))
                hst_g0 = load_group(0, split_first=True)
                nc.sync.dma_start(wv_sb[:], wv.ap().rearrange("(kt p) m -> p kt m", p=128))
                nc.sync.dma_start(wo_sb[:], wo[:])
                make_identity(nc, ident[:])
                # startup: k, q and v half-0 accumulate kt-interleaved so
                # each per-kt hst DMA is consumed as it lands (v borrows a
                # C slot transiently; ctx00 allocates only after it frees)
                p_k0 = ps.tile([128, 512], F32, tag="B", bufs=2, name="p_k00")
                p_q0 = ps.tile([128, 512], F32, tag="B", bufs=2, name="p_q00")
                p_v0 = ps.tile([128, 512], F32, tag="C", bufs=2, name="p_v00")
                for kt in range(8):
                    nc.tensor.matmul(p_k0[:], wk_sb[:, kt, :],
                                     hst_g0[:, kt, 0:512],
                                     start=(kt == 0), stop=(kt == 7))
                    nc.tensor.matmul(p_q0[:], wq_sb[:, kt, :],
                                     hst_g0[:, kt, 0:512],
                                     start=(kt == 0), stop=(kt == 7))
                    nc.tensor.matmul(p_v0[:], wv_sb[:, kt, :],
                                     hst_g0[:, kt, 0:512],
                                     start=(kt == 0), stop=(kt == 7))
                nc.vector.tensor_copy(kT_sb[:, 0:512], p_k0[:])
                nc.vector.tensor_copy(qT_sb[:, 0:512], p_q0[:])
                vT_0 = sb.tile([128, 512], F16, tag="vT_t", bufs=2,
                               name="vT_t0_0s")
                nc.vector.tensor_copy(vT_0[:], p_v0[:])
                for c in range(4):
                    p_tr = ps.tile([128, 128], F16, tag="B", bufs=2,
                                   name=f"p_tr_s{c}")
                    nc.tensor.transpose(
                        p_tr[:], vT_0[:, c * 128:(c + 1) * 128], ident[:])
                    v65_4d = v65.rearrange("p (pr kt) c -> p pr kt c", pr=4)
                    dst = v65_4d[:, 0:2, c, 0:64]
                    nc.vector.tensor_copy(
                        dst, p_tr.rearrange("p (h d) -> p h d", h=2))
                ctx00 = ctx_tiles(0, 0)
                attn_part(0, 0, range(0, 4), ctx00)
                proj_half("k", 0, 1, hst_g0)
                proj_half("v", 0, 1, hst_g0)
                proj_half("q", 0, 1, hst_g0)
                attn_part(0, 0, range(4, 8), ctx00)
                proj_group(1, load_group(1))
                attn_part(0, 0, range(8, KT), ctx00)
                norm(0, 0, ctx00)
                attn_block(0, 1, [])
                proj_group(2, load_group(2))
                attn_block(0, 2, [0, 1, 2, 3])
                proj_group(3, load_group(3))
                attn_block(0, 3, [4, 5, 6, 7])
                attn_block(1, 0, [8, 9, 10, 11])
                attn_block(1, 1, [12, 13, 14, 15])
                attn_block(1, 2, [16, 17, 18, 19])
                attn_block(1, 3, [20, 21, 22, 23, 24, 25, 26, 27])
                for qt in range(28, 32):
                    out_qt(qt)

            if n_iters == 1:
                for _u in range(unroll):
                    emit_body()
            else:
                with tc.For_i(0, n_iters, 1):
                    for _u in range(unroll):
                        emit_body()

    nc.finalize()
    return nc


def _get_nc(n_iters=1, unroll=1):
    key = (n_iters, unroll)
    if key not in _nc_cache:
        _nc_cache[key] = _build(n_iters, unroll)
    return _nc_cache[key]


def prepare_in_maps(hidden_states, w_q, w_k, w_v, w_o):
    hs = np.asarray(hidden_states, dtype=np.float32).reshape(NSEQ, QUERY_DIM)
    hst = np.ascontiguousarray(hs.T).astype(np.float16)
    wqs = (np.asarray(w_q, dtype=np.float32) * SCALE).astype(np.float16)
    wk16 = np.asarray(w_k, dtype=np.float32).astype(np.float16)
    wv16 = np.asarray(w_v, dtype=np.float32).astype(np.float16)
    wo16 = np.asarray(w_o, dtype=np.float32).astype(np.float16)
    in_maps = []
    for d in range(N_CORES):
        cols = slice(d * FEAT, (d + 1) * FEAT)
        in_maps.append({
            "hst": hst,
            "wq": np.ascontiguousarray(wqs[:, cols]),
            "wk": np.ascontiguousarray(wk16[:, cols]),
            "wv": np.ascontiguousarray(wv16[:, cols]),
            "wo": np.ascontiguousarray(wo16[cols, :]),
        })
    return in_maps


def run_spmd(in_maps, n_iters=1, unroll=1, **kwargs):
    from concourse.bass_utils import run_bass_kernel_spmd
    return run_bass_kernel_spmd(_get_nc(n_iters, unroll), in_maps,
                                core_ids=list(range(N_CORES)), **kwargs)


def kernel(hidden_states, w_q, w_k, w_v, w_o, b_o):
    in_maps = prepare_in_maps(hidden_states, w_q, w_k, w_v, w_o)
    res = run_spmd(in_maps)
    acc = np.zeros((NSEQ, QUERY_DIM), dtype=np.float32)
    for r in res.results:
        acc += r["out"].astype(np.float32)
    acc += np.asarray(b_o, dtype=np.float32)
    return acc.reshape(B, S, QUERY_DIM)


# revision 12
# speedup vs baseline: 1.0709x; 1.0709x over previous
"""Trainium2 Bass kernel for nn_CrossAttention (self-attention, B=2, S=2048,
16 heads x 64 dim, d_model=1024).

Sharding: batch*heads across 8 cores -> each core owns 2 heads for both
batches. Each core receives the full (pre-transposed, fp16) hidden states and
its 128-column slice of w_q/w_k/w_v (w_q pre-scaled by 1/sqrt(64)) plus its
128-row slice of w_o. Cores emit fp16 partial outputs [4096, 1024] (the w_o
contraction over the core's 128 inner dims); the host sums the partials in
fp32 and adds b_o.

Per-core dataflow (all matmuls fp16 operands, fp32 PSUM accumulation):
  1. Q^T/K^T/V^T [128, 4096] = w.T @ hs^T   (feature-major projections)
  2. V^T -> V plain [seq, 64] per k-tile via PE transpose, ones column
     appended (softmax denominator rides the ctx matmul for free)
  3. per (batch, q-chunk): scores^T[k, q] for BOTH heads packed into one
     PSUM tile via tile_position row groups (the two K=64 matmuls run
     concurrently on the PE array), one exp per packed tile on ACT,
     ctx^T[d, q] += [v | 1].T @ p^T per head
  4. normalize: DVE reciprocal -> gpsimd partition_broadcast -> DVE multiply
  5. out[q, 1024] = ctx^T.T @ w_o, interleaved into the next attention block

PSUM bank budget (8 banks of 2KB):
  tag A (scores, [128,1024] f32 = 2 banks) x2 bufs          -> 4 banks
  tag C (ctx accumulators, [65,512] f32 = 1 bank) x2 bufs   -> 2 banks
  tag B (proj/transpose/out transients, 1 bank) x2 bufs     -> 2 banks
With ctx in its own tag, projection bursts never pin the ctx slots, so
attention (ACT-bound) and projections (PE-bound) overlap.
"""
import numpy as np

HEADS = 16
DIM_HEAD = 64
QUERY_DIM = 1024
SCALE = DIM_HEAD ** -0.5
B, S = 2, 2048
NSEQ = B * S              # 4096
N_CORES = 8
FEAT = 128                # 2 heads x 64 per core
KT = S // 128             # 16 k-tiles per batch

_nc_cache = {}


def _build(n_iters=1, unroll=1):
    import concourse.bass as bass
    import concourse.tile as tile
    from concourse import bacc, mybir
    from concourse.masks import make_identity

    F32 = mybir.dt.float32
    F16 = mybir.dt.float16
    AF = mybir.ActivationFunctionType

    nc = bacc.Bacc("TRN2", target_bir_lowering=False)

    hst = nc.dram_tensor("hst", [QUERY_DIM, NSEQ], F16, kind="ExternalInput")
    wq = nc.dram_tensor("wq", [QUERY_DIM, FEAT], F16, kind="ExternalInput")
    wk = nc.dram_tensor("wk", [QUERY_DIM, FEAT], F16, kind="ExternalInput")
    wv = nc.dram_tensor("wv", [QUERY_DIM, FEAT], F16, kind="ExternalInput")
    wo = nc.dram_tensor("wo", [FEAT, QUERY_DIM], F16, kind="ExternalInput")
    out = nc.dram_tensor("out", [NSEQ, QUERY_DIM], F16, kind="ExternalOutput")

    with tile.TileContext(nc) as tc:
        with (
            tc.tile_pool(name="sb", bufs=1) as sb,
            tc.tile_pool(name="ps", bufs=1, space="PSUM") as ps,
            tc.tile_pool(name="dr", bufs=1, space="DRAM") as dr,
        ):
            wq_sb = sb.tile([128, 8, FEAT], F16)
            wk_sb = sb.tile([128, 8, FEAT], F16)
            wv_sb = sb.tile([128, 8, FEAT], F16)
            wo_sb = sb.tile([128, QUERY_DIM], F16)
            ident = sb.tile([128, 128], F16)

            qT_sb = sb.tile([128, NSEQ], F16)
            kT_sb = sb.tile([128, NSEQ], F16)
            ctxn_sb = sb.tile([128, NSEQ], F16)
            v65 = sb.tile([128, 4 * KT, 65], F16)
            nc.gpsimd.memset(v65[:, :, 64:65], 1.0)

            hst_src = hst.ap().rearrange("(kt p) n -> p kt n", p=128)
            w_sbs = {"q": wq_sb, "k": wk_sb, "v": wv_sb}

            def proj_half(proj, g, half, hst_t):
                """One 512-col half of one projection for seq-group g.
                For v, also transposes the 4 fresh 128-col tiles into v65."""
                g0 = g * 1024
                h0 = half * 512
                vT_t = None
                if proj == "v":
                    vT_t = sb.tile([128, 512], F16, tag="vT_t", bufs=2,
                                   name=f"vT_t{g}_{half}")
                p_p = ps.tile([128, 512], F32, tag="B", bufs=2,
                              name=f"p_{proj}{g}_{half}")
                for kt in range(8):
                    nc.tensor.matmul(
                        p_p[:], w_sbs[proj][:, kt, :],
                        hst_t[:, kt, h0:h0 + 512],
                        start=(kt == 0), stop=(kt == 7),
                    )
                if proj == "q":
                    nc.vector.tensor_copy(qT_sb[:, g0 + h0:g0 + h0 + 512], p_p[:])
                elif proj == "k":
                    nc.vector.tensor_copy(kT_sb[:, g0 + h0:g0 + h0 + 512], p_p[:])
                else:
                    nc.vector.tensor_copy(vT_t[:], p_p[:])
                    for c in range(4):
                        ci = g * 8 + half * 4 + c
                        b_i, kt_loc = ci // 16, ci % 16
                        p_tr = ps.tile([128, 128], F16, tag="B", bufs=2,
                                       name=f"p_tr{ci}")
                        nc.tensor.transpose(
                            p_tr[:], vT_t[:, c * 128:(c + 1) * 128], ident[:])
                        # one strided copy fills both heads' v65 slots
                        v65_4d = v65.rearrange("p (pr kt) c -> p pr kt c", pr=4)
                        dst = v65_4d[:, b_i * 2:(b_i + 1) * 2, kt_loc, 0:64]
                        nc.vector.tensor_copy(
                            dst, p_tr.rearrange("p (h d) -> p h d", h=2))

            def load_group(g, split_first=False):
                hst_t = sb.tile([128, 8, 1024], F16, tag="hst_t", bufs=2,
                                name=f"hst_t{g}")
                g0 = g * 1024
                if split_first:
                    # per-kt DMAs for the first 512 cols: the startup
                    # projection chain consumes kt i as soon as DMA i lands
                    for kt in range(8):
                        nc.sync.dma_start(hst_t[:, kt, 0:512],
                                          hst_src[:, kt, g0:g0 + 512])
                    nc.sync.dma_start(hst_t[:, :, 512:1024],
                                      hst_src[:, :, g0 + 512:g0 + 1024])
                else:
                    nc.sync.dma_start(hst_t[:], hst_src[:, :, g0:g0 + 1024])
                return hst_t

            def proj_group(g, hst_t):
                for half in range(2):
                    for proj in ("k", "q", "v"):
                        proj_half(proj, g, half, hst_t)

            def out_qt(qt):
                """One 128-row tile of the final projection."""
                t0 = qt * 128
                o_sb = sb.tile([128, 1024], F16, tag="o_sb", bufs=6,
                               name=f"o_sb{qt}")
                for c in range(2):
                    p_o = ps.tile([128, 512], F32, tag="B", bufs=2,
                                  name=f"p_o{qt}_{c}")
                    nc.tensor.matmul(
                        p_o[:], ctxn_sb[:, t0:t0 + 128],
                        wo_sb[:, c * 512:(c + 1) * 512],
                        start=True, stop=True)
                    nc.vector.tensor_copy(o_sb[:, c * 512:(c + 1) * 512], p_o[:])
                nc.sync.dma_start(out[t0:t0 + 128, :], o_sb[:])

            def attn_part(b_i, cc, kts, p_ctx, out_qts=()):
                """Score+exp+ctx for kt in kts, (batch, 512-q-chunk cc),
                head-packed scores; out_qts interleaved."""
                s0 = b_i * S
                q0 = s0 + cc * 512
                oq = list(out_qts)
                kts = list(kts)
                # spread out-tiles across the kt steps, starting at kt>=1
                pop_at = set()
                if oq:
                    navail = max(len(kts) - 1, 1)
                    step = max(navail // len(oq), 1)
                    for i in range(len(oq)):
                        pop_at.add(kts[min(1 + i * step, len(kts) - 1)])
                for kt in kts:
                    k0 = s0 + kt * 128
                    p_s = ps.tile([128, 1024], F32, tag="A", bufs=2,
                                  name=f"p_s{b_i}_{cc}_{kt}")
                    # head-packed: head h uses PE row group h*64, writes its
                    # own PSUM bank within the shared tile
                    for h in range(2):
                        hp = slice(h * 64, (h + 1) * 64)
                        nc.tensor.matmul(
                            p_s[:, h * 512:(h + 1) * 512],
                            kT_sb[hp, k0:k0 + 128],
                            qT_sb[hp, q0:q0 + 512],
                            start=True, stop=True,
                            tile_position=(h * 64, 0),
                        )
                    pT = sb.tile([128, 1024], F16, tag="pT", bufs=12,
                                 name=f"pT{b_i}_{cc}_{kt}")
                    nc.scalar.activation(pT[:], p_s[:], AF.Exp)
                    for h in range(2):
                        nc.tensor.matmul(
                            p_ctx[h][:],
                            v65[:, (b_i * 2 + h) * 16 + kt, :],
                            pT[:, h * 512:(h + 1) * 512],
                            start=(kt == 0), stop=(kt == KT - 1),
                        )
                    if kt in pop_at and oq:
                        out_qt(oq.pop(0))
                while oq:
                    out_qt(oq.pop(0))

            def ctx_tiles(b_i, cc):
                return [ps.tile([65, 512], F32, tag="C", bufs=2,
                                name=f"p_ctx{b_i}_{cc}_{h}")
                        for h in range(2)]

            def norm(b_i, cc, p_ctx):
                s0 = b_i * S
                q0 = s0 + cc * 512
                for h in range(2):
                    hp = slice(h * 64, (h + 1) * 64)
                    pc = p_ctx[h]
                    recip = sb.tile([1, 512], F32, tag="recip", bufs=4,
                                    name=f"recip{b_i}_{cc}_{h}")
                    nc.vector.reciprocal(recip[:], pc[64:65, :])
                    r_dr = dr.tile([1, 512], F32, tag="r_dr", bufs=4,
                                   name=f"r_dr{b_i}_{cc}_{h}")
                    nc.sync.dma_start(r_dr[:], recip[:])
                    rbc_sb = sb.tile([64, 512], F32, tag="rbc", bufs=6,
                                     name=f"rbc{b_i}_{cc}_{h}")
                    nc.sync.dma_start(rbc_sb[:],
                                      r_dr[0:1, :].to_broadcast([64, 512]))
                    nc.vector.tensor_mul(
                        ctxn_sb[hp, q0:q0 + 512], pc[0:64, :], rbc_sb[:])

            def attn_block(b_i, cc, out_qts, ctx=None, kts=None):
                p_ctx = ctx if ctx is not None else ctx_tiles(b_i, cc)
                attn_part(b_i, cc, kts if kts is not None else range(KT),
                          p_ctx, out_qts)
                if kts is None or kts[-1] == KT - 1:
                    norm(b_i, cc, p_ctx)
                return p_ctx

            # ---- emission order. Program order defines dataflow (producers
            # strictly before consumers); Tile pulls later-emitted independent
            # work into engine idle slots. Fine-grained start: attention on
            # (0,0) begins after the first half-projections of group 0. Out
            # tiles lag their chunk by one block; ctx slots (tag C) free at
            # each norm so the next chunk's ctx never waits on proj psum. ----
            def emit_body():
                nc.sync.dma_start(wk_sb[:], wk.ap().rearrange("(kt p) m -> p kt m", p=128))
                nc.sync.dma_start(wq_sb[:], wq.ap().rearrange("(kt p) m -> p kt m", p=128))
                hst_g0 = load_group(0, split_first=True)
                nc.sync.dma_start(wv_sb[:], wv.ap().rearrange("(kt p) m -> p kt m", p=128))
                nc.sync.dma_start(wo_sb[:], wo[:])
                make_identity(nc, ident[:])
                # startup: k, q and v half-0 accumulate kt-interleaved so
                # each per-kt hst DMA is consumed as it lands (v borrows a
                # C slot transiently; ctx00 allocates only after it frees)
                p_k0 = ps.tile([128, 512], F32, tag="B", bufs=2, name="p_k00")
                p_q0 = ps.tile([128, 512], F32, tag="B", bufs=2, name="p_q00")
                p_v0 = ps.tile([128, 512], F32, tag="C", bufs=2, name="p_v00")
                for kt in range(8):
                    nc.tensor.matmul(p_k0[:], wk_sb[:, kt, :],
                                     hst_g0[:, kt, 0:512],
                                     start=(kt == 0), stop=(kt == 7))
                    nc.tensor.matmul(p_q0[:], wq_sb[:, kt, :],
                                     hst_g0[:, kt, 0:512],
                                     start=(kt == 0), stop=(kt == 7))
                    nc.tensor.matmul(p_v0[:], wv_sb[:, kt, :],
                                     hst_g0[:, kt, 0:512],
                                     start=(kt == 0), stop=(kt == 7))
                nc.vector.tensor_copy(kT_sb[:, 0:512], p_k0[:])
                nc.vector.tensor_copy(qT_sb[:, 0:512], p_q0[:])
                vT_0 = sb.tile([128, 512], F16, tag="vT_t", bufs=2,
                               name="vT_t0_0s")
                nc.vector.tensor_copy(vT_0[:], p_v0[:])
                for c in range(4):
                    p_tr = ps.tile([128, 128], F16, tag="B", bufs=2,
                                   name=f"p_tr_s{c}")
                    nc.tensor.transpose(
                        p_tr[:], vT_0[:, c * 128:(c + 1) * 128], ident[:])
                    v65_4d = v65.rearrange("p (pr kt) c -> p pr kt c", pr=4)
                    dst = v65_4d[:, 0:2, c, 0:64]
                    nc.vector.tensor_copy(
                        dst, p_tr.rearrange("p (h d) -> p h d", h=2))
                ctx00 = ctx_tiles(0, 0)
                attn_part(0, 0, range(0, 4), ctx00)
                proj_half("k", 0, 1, hst_g0)
                proj_half("v", 0, 1, hst_g0)
                proj_half("q", 0, 1, hst_g0)
                attn_part(0, 0, range(4, 8), ctx00)
                proj_group(1, load_group(1))
                attn_part(0, 0, range(8, KT), ctx00)
                norm(0, 0, ctx00)
                attn_block(0, 1, [])
                proj_group(2, load_group(2))
                attn_block(0, 2, [0, 1, 2, 3])
                proj_group(3, load_group(3))
                attn_block(0, 3, [4, 5, 6, 7])
                attn_block(1, 0, [8, 9, 10, 11])
                attn_block(1, 1, [12, 13, 14, 15])
                attn_block(1, 2, [16, 17, 18, 19])
                attn_block(1, 3, [20, 21, 22, 23, 24, 25, 26, 27])
                for qt in range(28, 32):
                    out_qt(qt)

            if n_iters == 1:
                for _u in range(unroll):
                    emit_body()
            else:
                with tc.For_i(0, n_iters, 1):
                    for _u in range(unroll):
                        emit_body()

    nc.finalize()
    return nc


def _get_nc(n_iters=1, unroll=1):
    key = (n_iters, unroll)
    if key not in _nc_cache:
        _nc_cache[key] = _build(n_iters, unroll)
    return _nc_cache[key]


def prepare_in_maps(hidden_states, w_q, w_k, w_v, w_o):
    hs = np.asarray(hidden_states, dtype=np.float32).reshape(NSEQ, QUERY_DIM)
    hst = np.ascontiguousarray(hs.T).astype(np.float16)
    wqs = (np.asarray(w_q, dtype=np.float32) * SCALE).astype(np.float16)
    wk16 = np.asarray(w_k, dtype=np.float32).astype(np.float16)
    wv16 = np.asarray(w_v, dtype=np.float32).astype(np.float16)
    wo16 = np.asarray(w_o, dtype=np.float32).astype(np.float16)
    in_maps = []
    for d in range(N_CORES):
        cols = slice(d * FEAT, (d + 1) * FEAT)
        in_maps.append({
            "hst": hst,
            "wq": np.ascontiguousarray(wqs[:, cols]),
            "wk": np.ascontiguousarray(wk16[:, cols]),
            "wv": np.ascontiguousarray(wv16[:, cols]),
            "wo": np.ascontiguousarray(wo16[cols, :]),
        })
    return in_maps


def run_spmd(in_maps, n_iters=1, unroll=1, **kwargs):
    from concourse.bass_utils import run_bass_kernel_spmd
    return run_bass_kernel_spmd(_get_nc(n_iters, unroll), in_maps,
                                core_ids=list(range(N_CORES)), **kwargs)


def kernel(hidden_states, w_q, w_k, w_v, w_o, b_o):
    in_maps = prepare_in_maps(hidden_states, w_q, w_k, w_v, w_o)
    res = run_spmd(in_maps)
    acc = np.zeros((NSEQ, QUERY_DIM), dtype=np.float32)
    for r in res.results:
        acc += r["out"].astype(np.float32)
    acc += np.asarray(b_o, dtype=np.float32)
    return acc.reshape(B, S, QUERY_DIM)


# revision 14
# speedup vs baseline: 1.0872x; 1.0152x over previous
"""Trainium2 Bass kernel for nn_CrossAttention (self-attention, B=2, S=2048,
16 heads x 64 dim, d_model=1024).

Sharding: batch*heads across 8 cores -> each core owns 2 heads for both
batches. Each core receives the full (pre-transposed, fp16) hidden states and
its 128-column slice of w_q/w_k/w_v (w_q pre-scaled by 1/sqrt(64)) plus its
128-row slice of w_o. Cores emit fp16 partial outputs [4096, 1024] (the w_o
contraction over the core's 128 inner dims); the host sums the partials in
fp32 and adds b_o.

Per-core dataflow (all matmuls fp16 operands, fp32 PSUM accumulation):
  1. Q^T/K^T/V^T [128, 4096] = w.T @ hs^T   (feature-major projections)
  2. V^T -> V plain [seq, 64] per k-tile via PE transpose, ones column
     appended (softmax denominator rides the ctx matmul for free)
  3. per (batch, q-chunk): scores^T[k, q] for BOTH heads packed into one
     PSUM tile via tile_position row groups (the two K=64 matmuls run
     concurrently on the PE array), one exp per packed tile on ACT,
     ctx^T[d, q] += [v | 1].T @ p^T per head
  4. normalize: DVE reciprocal -> gpsimd partition_broadcast -> DVE multiply
  5. out[q, 1024] = ctx^T.T @ w_o, interleaved into the next attention block

PSUM bank budget (8 banks of 2KB):
  tag A (scores, [128,1024] f32 = 2 banks) x2 bufs          -> 4 banks
  tag C (ctx accumulators, [65,512] f32 = 1 bank) x2 bufs   -> 2 banks
  tag B (proj/transpose/out transients, 1 bank) x2 bufs     -> 2 banks
With ctx in its own tag, projection bursts never pin the ctx slots, so
attention (ACT-bound) and projections (PE-bound) overlap.
"""
import numpy as np

HEADS = 16
DIM_HEAD = 64
QUERY_DIM = 1024
SCALE = DIM_HEAD ** -0.5
B, S = 2, 2048
NSEQ = B * S              # 4096
N_CORES = 8
FEAT = 128                # 2 heads x 64 per core
KT = S // 128             # 16 k-tiles per batch

_nc_cache = {}


def _build(n_iters=1, unroll=1):
    import concourse.bass as bass
    import concourse.tile as tile
    from concourse import bacc, mybir
    from concourse.masks import make_identity

    F32 = mybir.dt.float32
    F16 = mybir.dt.float16
    AF = mybir.ActivationFunctionType

    nc = bacc.Bacc("TRN2", target_bir_lowering=False)

    hst = nc.dram_tensor("hst", [QUERY_DIM, NSEQ], F16, kind="ExternalInput")
    wq = nc.dram_tensor("wq", [QUERY_DIM, FEAT], F16, kind="ExternalInput")
    wk = nc.dram_tensor("wk", [QUERY_DIM, FEAT], F16, kind="ExternalInput")
    wv = nc.dram_tensor("wv", [QUERY_DIM, FEAT], F16, kind="ExternalInput")
    wo = nc.dram_tensor("wo", [FEAT, QUERY_DIM], F16, kind="ExternalInput")
    out = nc.dram_tensor("out", [NSEQ, QUERY_DIM], F16, kind="ExternalOutput")

    with tile.TileContext(nc) as tc:
        with (
            tc.tile_pool(name="sb", bufs=1) as sb,
            tc.tile_pool(name="ps", bufs=1, space="PSUM") as ps,
            tc.tile_pool(name="dr", bufs=1, space="DRAM") as dr,
        ):
            wq_sb = sb.tile([128, 8, FEAT], F16)
            wk_sb = sb.tile([128, 8, FEAT], F16)
            wv_sb = sb.tile([128, 8, FEAT], F16)
            wo_sb = sb.tile([128, QUERY_DIM], F16)
            ident = sb.tile([128, 128], F16)

            qT_sb = sb.tile([128, NSEQ], F16)
            kT_sb = sb.tile([128, NSEQ], F16)
            ctxn_sb = sb.tile([128, NSEQ], F16)
            v65 = sb.tile([128, 4 * KT, 65], F16)
            nc.gpsimd.memset(v65[:, :, 64:65], 1.0)

            hst_src = hst.ap().rearrange("(kt p) n -> p kt n", p=128)
            w_sbs = {"q": wq_sb, "k": wk_sb, "v": wv_sb}

            def proj_half(proj, g, half, hst_t):
                """One 512-col half of one projection for seq-group g.
                For v, also transposes the 4 fresh 128-col tiles into v65."""
                g0 = g * 1024
                h0 = half * 512
                vT_t = None
                if proj == "v":
                    vT_t = sb.tile([128, 512], F16, tag="vT_t", bufs=2,
                                   name=f"vT_t{g}_{half}")
                p_p = ps.tile([128, 512], F32, tag="B", bufs=2,
                              name=f"p_{proj}{g}_{half}")
                for kt in range(8):
                    nc.tensor.matmul(
                        p_p[:], w_sbs[proj][:, kt, :],
                        hst_t[:, kt, h0:h0 + 512],
                        start=(kt == 0), stop=(kt == 7),
                    )
                if proj == "q":
                    nc.vector.tensor_copy(qT_sb[:, g0 + h0:g0 + h0 + 512], p_p[:])
                elif proj == "k":
                    nc.vector.tensor_copy(kT_sb[:, g0 + h0:g0 + h0 + 512], p_p[:])
                else:
                    nc.vector.tensor_copy(vT_t[:], p_p[:])
                    for c in range(4):
                        ci = g * 8 + half * 4 + c
                        b_i, kt_loc = ci // 16, ci % 16
                        p_tr = ps.tile([128, 128], F16, tag="B", bufs=2,
                                       name=f"p_tr{ci}")
                        nc.tensor.transpose(
                            p_tr[:], vT_t[:, c * 128:(c + 1) * 128], ident[:])
                        # one strided copy fills both heads' v65 slots
                        v65_4d = v65.rearrange("p (pr kt) c -> p pr kt c", pr=4)
                        dst = v65_4d[:, b_i * 2:(b_i + 1) * 2, kt_loc, 0:64]
                        nc.vector.tensor_copy(
                            dst, p_tr.rearrange("p (h d) -> p h d", h=2))

            def load_group(g, split_first=False):
                hst_t = sb.tile([128, 8, 1024], F16, tag="hst_t", bufs=2,
                                name=f"hst_t{g}")
                g0 = g * 1024
                if split_first:
                    # per-kt DMAs for the first 512 cols: the startup
                    # projection chain consumes kt i as soon as DMA i lands
                    for kt in range(8):
                        nc.sync.dma_start(hst_t[:, kt, 0:512],
                                          hst_src[:, kt, g0:g0 + 512])
                    nc.sync.dma_start(hst_t[:, :, 512:1024],
                                      hst_src[:, :, g0 + 512:g0 + 1024])
                else:
                    nc.sync.dma_start(hst_t[:], hst_src[:, :, g0:g0 + 1024])
                return hst_t

            def proj_group(g, hst_t):
                for half in range(2):
                    for proj in ("k", "q", "v"):
                        proj_half(proj, g, half, hst_t)

            def out_qt(qt):
                """One 128-row tile of the final projection."""
                t0 = qt * 128
                o_sb = sb.tile([128, 1024], F16, tag="o_sb", bufs=6,
                               name=f"o_sb{qt}")
                for c in range(2):
                    p_o = ps.tile([128, 512], F32, tag="B", bufs=2,
                                  name=f"p_o{qt}_{c}")
                    nc.tensor.matmul(
                        p_o[:], ctxn_sb[:, t0:t0 + 128],
                        wo_sb[:, c * 512:(c + 1) * 512],
                        start=True, stop=True)
                    nc.vector.tensor_copy(o_sb[:, c * 512:(c + 1) * 512], p_o[:])
                nc.sync.dma_start(out[t0:t0 + 128, :], o_sb[:])

            def attn_part(b_i, cc, kts, p_ctx, out_qts=()):
                """Score+exp+ctx for kt in kts, (batch, 512-q-chunk cc),
                head-packed scores; out_qts interleaved."""
                s0 = b_i * S
                q0 = s0 + cc * 512
                oq = list(out_qts)
                kts = list(kts)
                # spread out-tiles across the second half of the kt steps:
                # out tiles read ctxn of the previous chunk, whose normalize
                # chain (DMA round-trip) completes a few us into this block
                pop_at = set()
                if oq:
                    lo = len(kts) // 2
                    navail = max(len(kts) - lo, 1)
                    step = max(navail // len(oq), 1)
                    for i in range(len(oq)):
                        pop_at.add(kts[min(lo + i * step, len(kts) - 1)])
                for kt in kts:
                    k0 = s0 + kt * 128
                    p_s = ps.tile([128, 1024], F32, tag="A", bufs=2,
                                  name=f"p_s{b_i}_{cc}_{kt}")
                    # head-packed: head h uses PE row group h*64, writes its
                    # own PSUM bank within the shared tile
                    for h in range(2):
                        hp = slice(h * 64, (h + 1) * 64)
                        nc.tensor.matmul(
                            p_s[:, h * 512:(h + 1) * 512],
                            kT_sb[hp, k0:k0 + 128],
                            qT_sb[hp, q0:q0 + 512],
                            start=True, stop=True,
                            tile_position=(h * 64, 0),
                        )
                    pT = sb.tile([128, 1024], F16, tag="pT", bufs=12,
                                 name=f"pT{b_i}_{cc}_{kt}")
                    nc.scalar.activation(pT[:], p_s[:], AF.Exp)
                    for h in range(2):
                        nc.tensor.matmul(
                            p_ctx[h][:],
                            v65[:, (b_i * 2 + h) * 16 + kt, :],
                            pT[:, h * 512:(h + 1) * 512],
                            start=(kt == 0), stop=(kt == KT - 1),
                        )
                    if kt in pop_at and oq:
                        out_qt(oq.pop(0))
                while oq:
                    out_qt(oq.pop(0))

            def ctx_tiles(b_i, cc):
                return [ps.tile([65, 512], F32, tag="C", bufs=2,
                                name=f"p_ctx{b_i}_{cc}_{h}")
                        for h in range(2)]

            def norm(b_i, cc, p_ctx):
                s0 = b_i * S
                q0 = s0 + cc * 512
                for h in range(2):
                    hp = slice(h * 64, (h + 1) * 64)
                    pc = p_ctx[h]
                    # copy out of PSUM first: frees the C slot in ~1us so the
                    # next chunk's ctx accumulation never waits on the (slow,
                    # DMA-round-trip) normalize chain below
                    ctxs = sb.tile([65, 512], F32, tag="ctxs", bufs=4,
                                   name=f"ctxs{b_i}_{cc}_{h}")
                    nc.vector.tensor_copy(ctxs[:], pc[:])
                    recip = sb.tile([1, 512], F32, tag="recip", bufs=4,
                                    name=f"recip{b_i}_{cc}_{h}")
                    nc.vector.reciprocal(recip[:], ctxs[64:65, :])
                    r_dr = dr.tile([1, 512], F32, tag="r_dr", bufs=4,
                                   name=f"r_dr{b_i}_{cc}_{h}")
                    nc.sync.dma_start(r_dr[:], recip[:])
                    rbc_sb = sb.tile([64, 512], F32, tag="rbc", bufs=6,
                                     name=f"rbc{b_i}_{cc}_{h}")
                    nc.sync.dma_start(rbc_sb[:],
                                      r_dr[0:1, :].to_broadcast([64, 512]))
                    nc.vector.tensor_mul(
                        ctxn_sb[hp, q0:q0 + 512], ctxs[0:64, :], rbc_sb[:])

            def attn_block(b_i, cc, out_qts, ctx=None, kts=None):
                p_ctx = ctx if ctx is not None else ctx_tiles(b_i, cc)
                attn_part(b_i, cc, kts if kts is not None else range(KT),
                          p_ctx, out_qts)
                if kts is None or kts[-1] == KT - 1:
                    norm(b_i, cc, p_ctx)
                return p_ctx

            # ---- emission order. Program order defines dataflow (producers
            # strictly before consumers); Tile pulls later-emitted independent
            # work into engine idle slots. Fine-grained start: attention on
            # (0,0) begins after the first half-projections of group 0. Out
            # tiles lag their chunk by one block; ctx slots (tag C) free at
            # each norm so the next chunk's ctx never waits on proj psum. ----
            def emit_body():
                nc.sync.dma_start(wk_sb[:], wk.ap().rearrange("(kt p) m -> p kt m", p=128))
                nc.sync.dma_start(wq_sb[:], wq.ap().rearrange("(kt p) m -> p kt m", p=128))
                hst_g0 = load_group(0, split_first=True)
                nc.sync.dma_start(wv_sb[:], wv.ap().rearrange("(kt p) m -> p kt m", p=128))
                nc.sync.dma_start(wo_sb[:], wo[:])
                make_identity(nc, ident[:])
                # startup: k, q and v half-0 accumulate kt-interleaved so
                # each per-kt hst DMA is consumed as it lands (v borrows a
                # C slot transiently; ctx00 allocates only after it frees)
                p_k0 = ps.tile([128, 512], F32, tag="B", bufs=2, name="p_k00")
                p_q0 = ps.tile([128, 512], F32, tag="B", bufs=2, name="p_q00")
                p_v0 = ps.tile([128, 512], F32, tag="C", bufs=2, name="p_v00")
                for kt in range(8):
                    nc.tensor.matmul(p_k0[:], wk_sb[:, kt, :],
                                     hst_g0[:, kt, 0:512],
                                     start=(kt == 0), stop=(kt == 7))
                    nc.tensor.matmul(p_q0[:], wq_sb[:, kt, :],
                                     hst_g0[:, kt, 0:512],
                                     start=(kt == 0), stop=(kt == 7))
                    nc.tensor.matmul(p_v0[:], wv_sb[:, kt, :],
                                     hst_g0[:, kt, 0:512],
                                     start=(kt == 0), stop=(kt == 7))
                nc.vector.tensor_copy(kT_sb[:, 0:512], p_k0[:])
                nc.vector.tensor_copy(qT_sb[:, 0:512], p_q0[:])
                vT_0 = sb.tile([128, 512], F16, tag="vT_t", bufs=2,
                               name="vT_t0_0s")
                nc.vector.tensor_copy(vT_0[:], p_v0[:])
                for c in range(4):
                    p_tr = ps.tile([128, 128], F16, tag="B", bufs=2,
                                   name=f"p_tr_s{c}")
                    nc.tensor.transpose(
                        p_tr[:], vT_0[:, c * 128:(c + 1) * 128], ident[:])
                    v65_4d = v65.rearrange("p (pr kt) c -> p pr kt c", pr=4)
                    dst = v65_4d[:, 0:2, c, 0:64]
                    nc.vector.tensor_copy(
                        dst, p_tr.rearrange("p (h d) -> p h d", h=2))
                ctx00 = ctx_tiles(0, 0)
                attn_part(0, 0, range(0, 4), ctx00)
                proj_half("k", 0, 1, hst_g0)
                proj_half("v", 0, 1, hst_g0)
                proj_half("q", 0, 1, hst_g0)
                attn_part(0, 0, range(4, 8), ctx00)
                proj_group(1, load_group(1))
                attn_part(0, 0, range(8, KT), ctx00)
                norm(0, 0, ctx00)
                attn_block(0, 1, [])
                proj_group(2, load_group(2))
                attn_block(0, 2, [0, 1, 2, 3])
                proj_group(3, load_group(3))
                attn_block(0, 3, [4, 5, 6, 7])
                attn_block(1, 0, [8, 9, 10, 11])
                attn_block(1, 1, [12, 13, 14, 15])
                attn_block(1, 2, [16, 17, 18, 19])
                attn_block(1, 3, [20, 21, 22, 23, 24, 25, 26, 27])
                for qt in range(28, 32):
                    out_qt(qt)

            if n_iters == 1:
                for _u in range(unroll):
                    emit_body()
            else:
                with tc.For_i(0, n_iters, 1):
                    for _u in range(unroll):
                        emit_body()

    nc.finalize()
    return nc


def _get_nc(n_iters=1, unroll=1):
    key = (n_iters, unroll)
    if key not in _nc_cache:
        _nc_cache[key] = _build(n_iters, unroll)
    return _nc_cache[key]


def prepare_in_maps(hidden_states, w_q, w_k, w_v, w_o):
    hs = np.asarray(hidden_states, dtype=np.float32).reshape(NSEQ, QUERY_DIM)
    hst = np.ascontiguousarray(hs.T).astype(np.float16)
    wqs = (np.asarray(w_q, dtype=np.float32) * SCALE).astype(np.float16)
    wk16 = np.asarray(w_k, dtype=np.float32).astype(np.float16)
    wv16 = np.asarray(w_v, dtype=np.float32).astype(np.float16)
    wo16 = np.asarray(w_o, dtype=np.float32).astype(np.float16)
    in_maps = []
    for d in range(N_CORES):
        cols = slice(d * FEAT, (d + 1) * FEAT)
        in_maps.append({
            "hst": hst,
            "wq": np.ascontiguousarray(wqs[:, cols]),
            "wk": np.ascontiguousarray(wk16[:, cols]),
            "wv": np.ascontiguousarray(wv16[:, cols]),
            "wo": np.ascontiguousarray(wo16[cols, :]),
        })
    return in_maps


def run_spmd(in_maps, n_iters=1, unroll=1, **kwargs):
    from concourse.bass_utils import run_bass_kernel_spmd
    return run_bass_kernel_spmd(_get_nc(n_iters, unroll), in_maps,
                                core_ids=list(range(N_CORES)), **kwargs)


def kernel(hidden_states, w_q, w_k, w_v, w_o, b_o):
    in_maps = prepare_in_maps(hidden_states, w_q, w_k, w_v, w_o)
    res = run_spmd(in_maps)
    acc = np.zeros((NSEQ, QUERY_DIM), dtype=np.float32)
    for r in res.results:
        acc += r["out"].astype(np.float32)
    acc += np.asarray(b_o, dtype=np.float32)
    return acc.reshape(B, S, QUERY_DIM)


# revision 23
# speedup vs baseline: 1.1239x; 1.0337x over previous
"""Trainium2 Bass kernel for nn_CrossAttention (self-attention, B=2, S=2048,
16 heads x 64 dim, d_model=1024).

Sharding: batch*heads across 8 cores -> each core owns 2 heads for both
batches. Each core receives the full (pre-transposed, fp16) hidden states and
its 128-column slice of w_q/w_k/w_v (w_q pre-scaled by 1/sqrt(64)) plus its
128-row slice of w_o. Cores emit fp16 partial outputs [4096, 1024] (the w_o
contraction over the core's 128 inner dims); the host sums the partials in
fp32 and adds b_o.

Per-core dataflow (all matmuls fp16 operands, fp32 PSUM accumulation):
  1. Q^T/K^T/V^T [128, 4096] = w.T @ hs^T   (feature-major projections)
  2. V^T -> V plain [seq, 64] per k-tile via PE transpose, ones column
     appended (softmax denominator rides the ctx matmul for free)
  3. per (batch, q-chunk): scores^T[k, q] for BOTH heads packed into one
     PSUM tile via tile_position row groups (the two K=64 matmuls run
     concurrently on the PE array), one exp per packed tile on ACT,
     ctx^T[d, q] += [v | 1].T @ p^T per head
  4. normalize: DVE reciprocal -> gpsimd partition_broadcast -> DVE multiply
  5. out[q, 1024] = ctx^T.T @ w_o, interleaved into the next attention block

PSUM bank budget (8 banks of 2KB):
  tag A (scores, [128,1024] f32 = 2 banks) x2 bufs          -> 4 banks
  tag C (ctx accumulators, [65,512] f32 = 1 bank) x2 bufs   -> 2 banks
  tag B (proj/transpose/out transients, 1 bank) x2 bufs     -> 2 banks
With ctx in its own tag, projection bursts never pin the ctx slots, so
attention (ACT-bound) and projections (PE-bound) overlap.
"""
import numpy as np

HEADS = 16
DIM_HEAD = 64
QUERY_DIM = 1024
SCALE = DIM_HEAD ** -0.5
B, S = 2, 2048
NSEQ = B * S              # 4096
N_CORES = 8
FEAT = 128                # 2 heads x 64 per core
KT = S // 128             # 16 k-tiles per batch

_nc_cache = {}


def _build(n_iters=1, unroll=1):
    import concourse.bass as bass
    import concourse.tile as tile
    from concourse import bacc, mybir
    from concourse.masks import make_identity

    F32 = mybir.dt.float32
    F16 = mybir.dt.float16
    AF = mybir.ActivationFunctionType

    nc = bacc.Bacc("TRN2", target_bir_lowering=False)

    hst = nc.dram_tensor("hst", [QUERY_DIM, NSEQ], F16, kind="ExternalInput")
    wq = nc.dram_tensor("wq", [QUERY_DIM, FEAT], F16, kind="ExternalInput")
    wk = nc.dram_tensor("wk", [QUERY_DIM, FEAT], F16, kind="ExternalInput")
    wv = nc.dram_tensor("wv", [QUERY_DIM, FEAT], F16, kind="ExternalInput")
    wo = nc.dram_tensor("wo", [FEAT, QUERY_DIM], F16, kind="ExternalInput")
    out = nc.dram_tensor("out", [NSEQ, QUERY_DIM], F16, kind="ExternalOutput")

    with tile.TileContext(nc) as tc:
        with (
            tc.tile_pool(name="sb", bufs=1) as sb,
            tc.tile_pool(name="ps", bufs=1, space="PSUM") as ps,
            tc.tile_pool(name="dr", bufs=1, space="DRAM") as dr,
        ):
            wq_sb = sb.tile([128, 8, FEAT], F16)
            wk_sb = sb.tile([128, 8, FEAT], F16)
            wv_sb = sb.tile([128, 8, FEAT], F16)
            wo_sb = sb.tile([128, QUERY_DIM], F16)
            ident = sb.tile([128, 128], F16)

            qT_sb = sb.tile([128, NSEQ], F16)
            kT_sb = sb.tile([128, NSEQ], F16)
            ctxn_sb = sb.tile([128, NSEQ], F16)
            v65 = sb.tile([128, 4 * KT, 65], F16)
            # DVE memset (not gpsimd): keeps the Pool engine exclusively on
            # attn-library ops (partition_broadcast) -> no Q7 ucode reloads.
            # Whole-tile memset so the seam warm-up matmuls below read
            # initialized data even on the first iteration.
            nc.vector.memset(v65[:], 0.0)
            nc.vector.memset(v65[:, :, 64:65], 1.0)
            # pre-loop dummy exp loads the ACT Exp table outside the For_i
            # body: with Exp live on both loop edges, insert_act_table_loads
            # can skip the per-iteration reload (1.3us/iter on the ACT pole)
            warm_exp = sb.tile([1, 64], F16)
            nc.scalar.activation(warm_exp[:], v65[0:1, 0, 0:64], AF.Exp)

            hst_src = hst.ap().rearrange("(kt p) n -> p kt n", p=128)
            w_sbs = {"q": wq_sb, "k": wk_sb, "v": wv_sb}

            def proj_half(proj, g, half, hst_t):
                """One 512-col half of one projection for seq-group g.
                For v, also transposes the 4 fresh 128-col tiles into v65."""
                g0 = g * 1024
                h0 = half * 512
                vT_t = None
                if proj == "v":
                    vT_t = sb.tile([128, 512], F16, tag="vT_t", bufs=2,
                                   name=f"vT_t{g}_{half}")
                p_p = ps.tile([128, 512], F32, tag="B", bufs=2,
                              name=f"p_{proj}{g}_{half}")
                for kt in range(8):
                    nc.tensor.matmul(
                        p_p[:], w_sbs[proj][:, kt, :],
                        hst_t[:, kt, h0:h0 + 512],
                        start=(kt == 0), stop=(kt == 7),
                    )
                if proj == "q":
                    nc.vector.tensor_copy(qT_sb[:, g0 + h0:g0 + h0 + 512], p_p[:])
                elif proj == "k":
                    nc.vector.tensor_copy(kT_sb[:, g0 + h0:g0 + h0 + 512], p_p[:])
                else:
                    nc.vector.tensor_copy(vT_t[:], p_p[:])
                    for c in range(4):
                        ci = g * 8 + half * 4 + c
                        b_i, kt_loc = ci // 16, ci % 16
                        p_tr = ps.tile([128, 128], F16, tag="B", bufs=2,
                                       name=f"p_tr{ci}")
                        nc.tensor.transpose(
                            p_tr[:], vT_t[:, c * 128:(c + 1) * 128], ident[:])
                        # one strided copy fills both heads' v65 slots
                        v65_4d = v65.rearrange("p (pr kt) c -> p pr kt c", pr=4)
                        dst = v65_4d[:, b_i * 2:(b_i + 1) * 2, kt_loc, 0:64]
                        nc.vector.tensor_copy(
                            dst, p_tr.rearrange("p (h d) -> p h d", h=2))

            def load_group(g, split_first=False):
                hst_t = sb.tile([128, 8, 1024], F16, tag="hst_t", bufs=3,
                                name=f"hst_t{g}")
                g0 = g * 1024
                if split_first:
                    # per-kt DMAs for the first 512 cols: the startup
                    # projection chain consumes kt i as soon as DMA i lands
                    for kt in range(8):
                        nc.sync.dma_start(hst_t[:, kt, 0:512],
                                          hst_src[:, kt, g0:g0 + 512])
                    nc.sync.dma_start(hst_t[:, :, 512:1024],
                                      hst_src[:, :, g0 + 512:g0 + 1024])
                else:
                    nc.sync.dma_start(hst_t[:], hst_src[:, :, g0:g0 + 1024])
                return hst_t

            def proj_group(g, hst_t):
                for half in range(2):
                    for proj in ("k", "q", "v"):
                        proj_half(proj, g, half, hst_t)

            def out_qt(qt, queue="sync"):
                """One 128-row tile of the final projection."""
                t0 = qt * 128
                o_sb = sb.tile([128, 1024], F16, tag="o_sb", bufs=6,
                               name=f"o_sb{qt}")
                for c in range(2):
                    p_o = ps.tile([128, 512], F32, tag="B", bufs=2,
                                  name=f"p_o{qt}_{c}")
                    nc.tensor.matmul(
                        p_o[:], ctxn_sb[:, t0:t0 + 128],
                        wo_sb[:, c * 512:(c + 1) * 512],
                        start=True, stop=True)
                    nc.vector.tensor_copy(o_sb[:, c * 512:(c + 1) * 512], p_o[:])
                eng = nc.scalar if queue == "scalar" else nc.sync
                eng.dma_start(out[t0:t0 + 128, :], o_sb[:])

            def attn_part(b_i, cc, kts, p_ctx, out_qts=()):
                """Score+exp+ctx for kt in kts, (batch, 512-q-chunk cc),
                head-packed scores; out_qts interleaved."""
                s0 = b_i * S
                q0 = s0 + cc * 512
                oq = list(out_qts)
                kts = list(kts)
                # spread out-tiles across the second half of the kt steps:
                # out tiles read ctxn of the previous chunk, whose normalize
                # chain (DMA round-trip) completes a few us into this block
                pop_at = set()
                if oq:
                    lo = len(kts) // 2
                    navail = max(len(kts) - lo, 1)
                    step = max(navail // len(oq), 1)
                    for i in range(len(oq)):
                        pop_at.add(kts[min(lo + i * step, len(kts) - 1)])
                for kt in kts:
                    k0 = s0 + kt * 128
                    p_s = ps.tile([128, 1024], F32, tag="A", bufs=2,
                                  name=f"p_s{b_i}_{cc}_{kt}")
                    # head-packed: head h uses PE row group h*64, writes its
                    # own PSUM bank within the shared tile
                    for h in range(2):
                        hp = slice(h * 64, (h + 1) * 64)
                        nc.tensor.matmul(
                            p_s[:, h * 512:(h + 1) * 512],
                            kT_sb[hp, k0:k0 + 128],
                            qT_sb[hp, q0:q0 + 512],
                            start=True, stop=True,
                            tile_position=(h * 64, 0),
                        )
                    pT = sb.tile([128, 1024], F16, tag="pT", bufs=12,
                                 name=f"pT{b_i}_{cc}_{kt}")
                    nc.scalar.activation(pT[:], p_s[:], AF.Exp)
                    for h in range(2):
                        nc.tensor.matmul(
                            p_ctx[h][:],
                            v65[:, (b_i * 2 + h) * 16 + kt, :],
                            pT[:, h * 512:(h + 1) * 512],
                            start=(kt == 0), stop=(kt == KT - 1),
                        )
                    if kt in pop_at and oq:
                        out_qt(oq.pop(0))
                while oq:
                    out_qt(oq.pop(0))

            def ctx_tiles(b_i, cc):
                return [ps.tile([65, 512], F32, tag="C", bufs=2,
                                name=f"p_ctx{b_i}_{cc}_{h}")
                        for h in range(2)]

            def norm(b_i, cc, p_ctx):
                s0 = b_i * S
                q0 = s0 + cc * 512
                for h in range(2):
                    hp = slice(h * 64, (h + 1) * 64)
                    pc = p_ctx[h]
                    # copy out of PSUM first: frees the C slot in ~1us so the
                    # next chunk's ctx accumulation never waits on the (slow,
                    # DMA-round-trip) normalize chain below
                    ctxs = sb.tile([65, 512], F32, tag="ctxs", bufs=4,
                                   name=f"ctxs{b_i}_{cc}_{h}")
                    nc.vector.tensor_copy(ctxs[:], pc[:])
                    recip = sb.tile([1, 512], F32, tag="recip", bufs=4,
                                    name=f"recip{b_i}_{cc}_{h}")
                    nc.vector.reciprocal(recip[:], ctxs[64:65, :])
                    rbc_sb = sb.tile([64, 512], F32, tag="rbc", bufs=6,
                                     name=f"rbc{b_i}_{cc}_{h}")
                    nc.gpsimd.partition_broadcast(rbc_sb[:], recip[:])
                    nc.vector.tensor_mul(
                        ctxn_sb[hp, q0:q0 + 512], ctxs[0:64, :], rbc_sb[:])

            def attn_block(b_i, cc, out_qts, ctx=None, kts=None):
                p_ctx = ctx if ctx is not None else ctx_tiles(b_i, cc)
                attn_part(b_i, cc, kts if kts is not None else range(KT),
                          p_ctx, out_qts)
                if kts is None or kts[-1] == KT - 1:
                    norm(b_i, cc, p_ctx)
                return p_ctx

            # ---- emission order. Program order defines dataflow (producers
            # strictly before consumers); Tile pulls later-emitted independent
            # work into engine idle slots. Fine-grained start: attention on
            # (0,0) begins after the first half-projections of group 0. Out
            # tiles lag their chunk by one block; ctx slots (tag C) free at
            # each norm so the next chunk's ctx never waits on proj psum. ----
            def emit_body():
                # seam warm-up: 8 matmuls on always-initialized v65 data keep
                # the PE HAM clock gate warm across the For_i loop seam (they
                # overlap the head DMAs; the PE would otherwise idle >3.4us
                # and re-throttle to 1.2 GHz for the next ~3.4us)
                v65_flat = v65.rearrange("p s c -> p (s c)")
                p_warm = ps.tile([128, 1024], F32, tag="A", bufs=2,
                                 name="p_warm")
                for _w in range(20):
                    nc.tensor.matmul(p_warm[0:65, 0:512], v65[:, 0, :],
                                     v65_flat[:, 0:512],
                                     start=True, stop=True)
                # weight DMAs issue from the (idle-at-head) ACT queue so they
                # don't serialize behind the hst loads on the SP queue
                nc.scalar.dma_start(wk_sb[:], wk.ap().rearrange("(kt p) m -> p kt m", p=128))
                nc.scalar.dma_start(wq_sb[:], wq.ap().rearrange("(kt p) m -> p kt m", p=128))
                hst_g0 = load_group(0, split_first=True)
                nc.scalar.dma_start(wv_sb[:], wv.ap().rearrange("(kt p) m -> p kt m", p=128))
                nc.scalar.dma_start(wo_sb[:], wo[:])
                make_identity(nc, ident[:])
                # startup: k, q and v half-0 accumulate kt-interleaved so
                # each per-kt hst DMA is consumed as it lands (v borrows a
                # C slot transiently; ctx00 allocates only after it frees)
                p_k0 = ps.tile([128, 512], F32, tag="B", bufs=2, name="p_k00")
                p_q0 = ps.tile([128, 512], F32, tag="B", bufs=2, name="p_q00")
                p_v0 = ps.tile([128, 512], F32, tag="C", bufs=2, name="p_v00")
                for kt in range(8):
                    nc.tensor.matmul(p_k0[:], wk_sb[:, kt, :],
                                     hst_g0[:, kt, 0:512],
                                     start=(kt == 0), stop=(kt == 7))
                    nc.tensor.matmul(p_q0[:], wq_sb[:, kt, :],
                                     hst_g0[:, kt, 0:512],
                                     start=(kt == 0), stop=(kt == 7))
                    nc.tensor.matmul(p_v0[:], wv_sb[:, kt, :],
                                     hst_g0[:, kt, 0:512],
                                     start=(kt == 0), stop=(kt == 7))
                nc.vector.tensor_copy(kT_sb[:, 0:512], p_k0[:])
                nc.vector.tensor_copy(qT_sb[:, 0:512], p_q0[:])
                # prefetch: queue the remaining hst group loads early on SP,
                # ahead of the out-store traffic (hst_t ring holds 3 groups)
                hst_g1 = load_group(1)
                vT_0 = sb.tile([128, 512], F16, tag="vT_t", bufs=2,
                               name="vT_t0_0s")
                nc.vector.tensor_copy(vT_0[:], p_v0[:])
                for c in range(4):
                    p_tr = ps.tile([128, 128], F16, tag="B", bufs=2,
                                   name=f"p_tr_s{c}")
                    nc.tensor.transpose(
                        p_tr[:], vT_0[:, c * 128:(c + 1) * 128], ident[:])
                    v65_4d = v65.rearrange("p (pr kt) c -> p pr kt c", pr=4)
                    dst = v65_4d[:, 0:2, c, 0:64]
                    nc.vector.tensor_copy(
                        dst, p_tr.rearrange("p (h d) -> p h d", h=2))
                ctx00 = ctx_tiles(0, 0)
                attn_part(0, 0, range(0, 4), ctx00)
                hst_g2 = load_group(2)
                proj_half("k", 0, 1, hst_g0)
                proj_half("v", 0, 1, hst_g0)
                proj_half("q", 0, 1, hst_g0)
                hst_g3 = load_group(3)
                attn_part(0, 0, range(4, 8), ctx00)
                proj_group(1, hst_g1)
                attn_part(0, 0, range(8, KT), ctx00)
                norm(0, 0, ctx00)
                attn_block(0, 1, [])
                proj_group(2, hst_g2)
                attn_block(0, 2, [0, 1, 2, 3])
                proj_group(3, hst_g3)
                attn_block(0, 3, [4, 5, 6, 7])
                attn_block(1, 0, [8, 9, 10, 11])
                attn_block(1, 1, [12, 13, 14, 15])
                attn_block(1, 2, [16, 17, 18, 19])
                attn_block(1, 3, [20, 21, 22, 23, 24, 25, 26, 27])
                for i, qt in enumerate(range(28, 32)):
                    out_qt(qt, queue=("scalar" if i % 2 else "sync"))

            if n_iters == 1:
                for _u in range(unroll):
                    emit_body()
            else:
                with tc.For_i(0, n_iters, 1):
                    for _u in range(unroll):
                        emit_body()

    nc.finalize()
    return nc


def _get_nc(n_iters=1, unroll=1):
    key = (n_iters, unroll)
    if key not in _nc_cache:
        _nc_cache[key] = _build(n_iters, unroll)
    return _nc_cache[key]


def prepare_in_maps(hidden_states, w_q, w_k, w_v, w_o):
    hs = np.asarray(hidden_states, dtype=np.float32).reshape(NSEQ, QUERY_DIM)
    hst = np.ascontiguousarray(hs.T).astype(np.float16)
    wqs = (np.asarray(w_q, dtype=np.float32) * SCALE).astype(np.float16)
    wk16 = np.asarray(w_k, dtype=np.float32).astype(np.float16)
    wv16 = np.asarray(w_v, dtype=np.float32).astype(np.float16)
    wo16 = np.asarray(w_o, dtype=np.float32).astype(np.float16)
    in_maps = []
    for d in range(N_CORES):
        cols = slice(d * FEAT, (d + 1) * FEAT)
        in_maps.append({
            "hst": hst,
            "wq": np.ascontiguousarray(wqs[:, cols]),
            "wk": np.ascontiguousarray(wk16[:, cols]),
            "wv": np.ascontiguousarray(wv16[:, cols]),
            "wo": np.ascontiguousarray(wo16[cols, :]),
        })
    return in_maps


def run_spmd(in_maps, n_iters=1, unroll=1, **kwargs):
    from concourse.bass_utils import run_bass_kernel_spmd
    return run_bass_kernel_spmd(_get_nc(n_iters, unroll), in_maps,
                                core_ids=list(range(N_CORES)), **kwargs)


def kernel(hidden_states, w_q, w_k, w_v, w_o, b_o):
    in_maps = prepare_in_maps(hidden_states, w_q, w_k, w_v, w_o)
    res = run_spmd(in_maps)
    acc = np.zeros((NSEQ, QUERY_DIM), dtype=np.float32)
    for r in res.results:
        acc += r["out"].astype(np.float32)
    acc += np.asarray(b_o, dtype=np.float32)
    return acc.reshape(B, S, QUERY_DIM)
